# revision 6
# baseline (speedup 1.0000x reference)
"""Attention-pooling kernel for TRN2 (8 NeuronCores, SPMD batch-parallel).

Problem: b=32, s=4096, h=1024 fp32
    scores[b,s] = sum_h dec[b,h] * enc[b,s,h]
    attn        = softmax(scores, axis=-1)
    context[b,h]= sum_s attn[b,s] * enc[b,s,h]

Strategy: shard batch over 8 cores (4 batches/core). Per batch, the 16 MiB
encoder block is streamed into SBUF once and both contractions read it from
SBUF, so HBM traffic is the 64 MiB/core floor (~186 us at ~358 GB/s).
  - scores: DVE tensor_tensor_reduce (enc_tile * dec_bcast, accum over h)
  - softmax: free-dim reduces + small PE transposes / ones-matmuls for the
    cross-partition max/sum/broadcast steps
  - context: PE matmuls lhsT=e[:,j] (K=128 s-rows, M=1), rhs=enc tile
    (N=512), accumulated in PSUM over the 32 s-subtiles
"""

import numpy as np

B, S, H = 32, 4096, 1024
NCORES = 8
B_LOC = B // NCORES  # 4 batches per core
P = 128              # partitions
F = 2                # s-subtiles per DMA (1 MiB loads)
NT = S // P          # 32 s-subtiles per batch
NBIG = NT // F       # 16 DMA'd tiles per batch
NH = H // 512        # moving-dim chunks per matmul row

_CACHE = {}


def build_program(finalize=True):
    import concourse.mybir as mybir
    import concourse.tile as tile
    from concourse import bacc
    from concourse.masks import make_identity

    f32 = mybir.dt.float32
    AF = mybir.ActivationFunctionType
    AX = mybir.AxisListType
    OP = mybir.AluOpType

    nc = bacc.Bacc("TRN2", target_bir_lowering=False, debug=False)
    dec = nc.dram_tensor("dec", [B_LOC, H], f32, kind="ExternalInput")
    enc = nc.dram_tensor("enc", [B_LOC, S, H], f32, kind="ExternalInput")
    ctx_out = nc.dram_tensor("context", [B_LOC, H], f32, kind="ExternalOutput")
    attn_out = nc.dram_tensor("attn", [B_LOC, S], f32, kind="ExternalOutput")

    with tile.TileContext(nc) as tc:
        with (
            tc.tile_pool(name="const", bufs=1) as const_pool,
            tc.tile_pool(name="encp", bufs=NBIG + 2) as enc_pool,
            tc.tile_pool(name="decp", bufs=2) as dec_pool,
            tc.tile_pool(name="scr", bufs=3) as scr_pool,
            tc.tile_pool(name="small", bufs=2) as small_pool,
            tc.tile_pool(name="psA", bufs=1, space="PSUM") as psA,
            tc.tile_pool(name="psT", bufs=1, space="PSUM") as psT,
            tc.tile_pool(name="psS", bufs=2, space="PSUM") as psS,
            tc.tile_pool(name="psC", bufs=1, space="PSUM") as psC,
        ):
            identity = const_pool.tile([P, P], f32)
            make_identity(nc, identity[:])
            ones_col = const_pool.tile([P, 1], f32)  # lhsT for partition-sum
            nc.vector.memset(ones_col[:], 1.0)
            ones_row = const_pool.tile([1, P], f32)  # lhsT for partition-bcast
            nc.vector.memset(ones_row[:], 1.0)

            for b in range(B_LOC):
                # --- broadcast dec[b] to all 128 partitions (PE ones-matmul)
                dec_row = dec_pool.tile([1, H], f32, tag="dec_row")
                nc.sync.dma_start(out=dec_row[:], in_=dec[b : b + 1, :])
                dec_ps = psA.tile([P, H], f32, tag="dec_ps")
                dec_b = dec_pool.tile([P, H], f32, tag="dec_b")
                for k in range(NH):
                    c = slice(k * 512, (k + 1) * 512)
                    nc.tensor.matmul(
                        dec_ps[:, c], ones_row[:], dec_row[:, c],
                        start=True, stop=True,
                    )
                    nc.scalar.copy(dec_b[:, c], dec_ps[:, c])

                # --- stream encoder tiles; scores via fused mult+reduce on DVE
                scores = small_pool.tile([P, NT], f32, tag="scores")
                bigs = []
                for jb in range(NBIG):
                    t = enc_pool.tile([P, F, H], f32, tag="enc")
                    nc.sync.dma_start(
                        out=t[:],
                        in_=enc[b, jb * (F * P) : (jb + 1) * (F * P), :].rearrange(
                            "(f p) h -> p f h", p=P
                        ),
                    )
                    for f in range(F):
                        jj = jb * F + f
                        # tensor_tensor_reduce would fuse these, but it
                        # crashes the exec unit on this runtime; split into
                        # DVE mult + ACT accumulate-copy (in-place).
                        scr = scr_pool.tile([P, H], f32, tag="scr")
                        nc.vector.tensor_mul(scr[:], t[:, f, :], dec_b[:])
                        nc.scalar.activation(
                            scr[:], scr[:], AF.Copy,
                            accum_out=scores[:, jj : jj + 1],
                        )
                    bigs.append(t)

                # --- softmax over [128 partitions x 32 cols]
                m_part = small_pool.tile([P, 1], f32, tag="m_part")
                nc.vector.reduce_max(m_part[:], scores[:], axis=AX.X)
                mT = psS.tile([1, P], f32, tag="psS")
                nc.tensor.transpose(mT[:], m_part[:], identity[:])
                negm = small_pool.tile([1, 1], f32, tag="negm")
                nc.vector.reduce_max(negm[:], mT[:], axis=AX.X, negate=True)
                negm_ps = psS.tile([P, 1], f32, tag="psS")
                nc.tensor.matmul(
                    negm_ps[:], ones_row[:], negm[:], start=True, stop=True
                )
                negm_b = small_pool.tile([P, 1], f32, tag="negm_b")
                nc.scalar.copy(negm_b[:], negm_ps[:])

                e = small_pool.tile([P, NT], f32, tag="e")
                z_part = small_pool.tile([P, 1], f32, tag="z_part")
                nc.scalar.activation(
                    e[:], scores[:], AF.Exp,
                    bias=negm_b[:], scale=1.0, accum_out=z_part[:],
                )
                z_ps = psS.tile([1, 1], f32, tag="psS")
                nc.tensor.matmul(z_ps[:], ones_col[:], z_part[:], start=True, stop=True)
                rz = small_pool.tile([1, 1], f32, tag="rz")
                nc.vector.reciprocal(rz[:], z_ps[:])
                rz_ps = psS.tile([P, 1], f32, tag="psS")
                nc.tensor.matmul(rz_ps[:], ones_row[:], rz[:], start=True, stop=True)
                rz_b = small_pool.tile([P, 1], f32, tag="rz_b")
                nc.scalar.copy(rz_b[:], rz_ps[:])

                # --- attn output, transposed to [32, 128] for contiguous rows
                eT = psT.tile([NT, P], f32, tag="eT")
                nc.tensor.transpose(eT[:], e[:], identity[:])
                attnT = small_pool.tile([NT, P], f32, tag="attnT")
                nc.scalar.mul(attnT[:], eT[:], rz_b[0:NT, :])
                nc.sync.dma_start(
                    out=attn_out[b].rearrange("(j p) -> j p", p=P), in_=attnT[:]
                )

                # --- context: accumulate e-weighted rows over all s-subtiles
                ctx_ps = psC.tile([1, H], f32, tag="ctx_ps")
                for jb in range(NBIG):
                    for f in range(F):
                        jj = jb * F + f
                        st = jj == 0
                        sp = jj == NT - 1
                        for k in range(NH):
                            c = slice(k * 512, (k + 1) * 512)
                            nc.tensor.matmul(
                                ctx_ps[:, c],
                                e[:, jj : jj + 1],
                                bigs[jb][:, f, c],
                                start=st, stop=sp,
                            )
                ctx_sb = small_pool.tile([1, H], f32, tag="ctx_sb")
                for k in range(NH):
                    c = slice(k * 512, (k + 1) * 512)
                    nc.scalar.mul(ctx_sb[:, c], ctx_ps[:, c], rz[:])
                nc.sync.dma_start(out=ctx_out[b : b + 1, :], in_=ctx_sb[:])

    if finalize:
        nc.finalize()
    return nc


def _get_program():
    if "nc" not in _CACHE:
        _CACHE["nc"] = build_program()
    return _CACHE["nc"]


def _install_ntff_hook_shim():
    """Provide antenv.axon_hooks if the image lacks it, so trace=True /
    BASS_TRACE=1 works (and doesn't crash run_bass_kernel_spmd)."""
    import sys
    import types

    try:
        import antenv.axon_hooks  # noqa: F401

        return
    except ImportError:
        pass
    hook = None
    try:
        from trn_agent_boot.trn_boot import _ntff_profile_via_ctypes

        hook = _ntff_profile_via_ctypes("/opt/axon/libaxon_pjrt.so")
    except Exception:
        pass
    mod = types.ModuleType("antenv.axon_hooks")
    state = {"hook": hook}
    mod.set_axon_ntff_profile_hook = lambda h: state.__setitem__("hook", h)
    mod.get_axon_ntff_profile_hook = lambda: state["hook"]
    sys.modules["antenv.axon_hooks"] = mod
    try:
        import antenv

        antenv.axon_hooks = mod
    except ImportError:
        pass


def kernel(decoder_state_t, encoder_outputs):
    _install_ntff_hook_shim()
    from concourse.bass_utils import run_bass_kernel_spmd

    nc = _get_program()
    dec = np.ascontiguousarray(np.asarray(decoder_state_t, dtype=np.float32))
    enc = np.ascontiguousarray(np.asarray(encoder_outputs, dtype=np.float32))
    assert dec.shape == (B, H) and enc.shape == (B, S, H)

    in_maps = []
    for c in range(NCORES):
        sl = slice(c * B_LOC, (c + 1) * B_LOC)
        in_maps.append({"dec": dec[sl], "enc": enc[sl]})

    res = run_bass_kernel_spmd(nc, in_maps, core_ids=list(range(NCORES)))
    _CACHE["last"] = res

    context = np.concatenate([r["context"] for r in res.results], axis=0)
    attn = np.concatenate([r["attn"] for r in res.results], axis=0)
    return (context, attn)


# revision 9
# speedup vs baseline: 1.2519x; 1.2519x over previous
"""Attention-pooling kernel for TRN2 (8 NeuronCores, SPMD batch-parallel).

Problem: b=32, s=4096, h=1024 fp32
    scores[b,s] = sum_h dec[b,h] * enc[b,s,h]
    attn        = softmax(scores, axis=-1)
    context[b,h]= sum_s attn[b,s] * enc[b,s,h]

Strategy: shard batch over 8 cores (4 batches/core). Per batch, the 16 MiB
encoder block is streamed into SBUF once and both contractions read it from
SBUF, so HBM traffic is the 64 MiB/core floor (~186 us at ~358 GB/s).
  - scores: DVE tensor_tensor_reduce (enc_tile * dec_bcast, accum over h)
  - softmax: free-dim reduces + small PE transposes / ones-matmuls for the
    cross-partition max/sum/broadcast steps
  - context: PE matmuls lhsT=e[:,j] (K=128 s-rows, M=1), rhs=enc tile
    (N=512), accumulated in PSUM over the 32 s-subtiles
"""

import numpy as np

B, S, H = 32, 4096, 1024
NCORES = 8
B_LOC = B // NCORES  # 4 batches per core
P = 128              # partitions
F = 2                # s-subtiles per DMA (1 MiB loads)
NT = S // P          # 32 s-subtiles per batch
NBIG = NT // F       # 16 DMA'd tiles per batch
NH = H // 512        # moving-dim chunks per matmul row

_CACHE = {}


def build_program(finalize=True):
    import concourse.mybir as mybir
    import concourse.tile as tile
    from concourse import bacc
    from concourse.masks import make_identity

    f32 = mybir.dt.float32
    AF = mybir.ActivationFunctionType
    AX = mybir.AxisListType
    OP = mybir.AluOpType

    nc = bacc.Bacc("TRN2", target_bir_lowering=False, debug=False)
    dec = nc.dram_tensor("dec", [B_LOC, H], f32, kind="ExternalInput")
    enc = nc.dram_tensor("enc", [B_LOC, S, H], f32, kind="ExternalInput")
    ctx_out = nc.dram_tensor("context", [B_LOC, H], f32, kind="ExternalOutput")
    attn_out = nc.dram_tensor("attn", [B_LOC, S], f32, kind="ExternalOutput")

    bf16 = mybir.dt.bfloat16

    with tile.TileContext(nc) as tc:
        with (
            tc.tile_pool(name="const", bufs=1) as const_pool,
            tc.tile_pool(name="encp", bufs=6) as enc_pool,
            tc.tile_pool(name="enc16p", bufs=NBIG + 2) as enc16_pool,
            tc.tile_pool(name="decp", bufs=2) as dec_pool,
            tc.tile_pool(name="scr", bufs=3) as scr_pool,
            tc.tile_pool(name="small", bufs=2) as small_pool,
            tc.tile_pool(name="psA", bufs=1, space="PSUM") as psA,
            tc.tile_pool(name="psT", bufs=1, space="PSUM") as psT,
            tc.tile_pool(name="psS", bufs=2, space="PSUM") as psS,
            tc.tile_pool(name="psC", bufs=1, space="PSUM") as psC,
        ):
            identity = const_pool.tile([P, P], f32)
            make_identity(nc, identity[:])
            ones_col = const_pool.tile([P, 1], f32)  # lhsT for partition-sum
            nc.vector.memset(ones_col[:], 1.0)
            ones_row = const_pool.tile([1, P], f32)  # lhsT for partition-bcast
            nc.vector.memset(ones_row[:], 1.0)

            for b in range(B_LOC):
                # --- broadcast dec[b] to all 128 partitions (PE ones-matmul)
                dec_row = dec_pool.tile([1, H], f32, tag="dec_row")
                nc.sync.dma_start(out=dec_row[:], in_=dec[b : b + 1, :])
                dec_ps = psA.tile([P, H], f32, tag="dec_ps")
                dec_b = dec_pool.tile([P, H], f32, tag="dec_b")
                for k in range(NH):
                    c = slice(k * 512, (k + 1) * 512)
                    nc.tensor.matmul(
                        dec_ps[:, c], ones_row[:], dec_row[:, c],
                        start=True, stop=True,
                    )
                    nc.scalar.copy(dec_b[:, c], dec_ps[:, c])

                # --- stream encoder tiles: fused dot-product on DVE for
                # scores (scalar_tensor_tensor with accum_out), bf16 copy on
                # ACT for the context matmuls (4x cheaper on PE than fp32's
                # LOW_HIGH two-pass mode; context error ~2e-3 scale-relative)
                scores = small_pool.tile([P, NT], f32, tag="scores")
                bigs = []
                for jb in range(NBIG):
                    t = enc_pool.tile([P, F, H], f32, tag="enc")
                    nc.sync.dma_start(
                        out=t[:],
                        in_=enc[b, jb * (F * P) : (jb + 1) * (F * P), :].rearrange(
                            "(f p) h -> p f h", p=P
                        ),
                    )
                    t16 = enc16_pool.tile([P, F, H], bf16, tag="enc16")
                    for f in range(F):
                        jj = jb * F + f
                        scr = scr_pool.tile([P, H], f32, tag="scr")
                        nc.vector.scalar_tensor_tensor(
                            out=scr[:], in0=t[:, f, :], scalar=1.0, in1=dec_b[:],
                            op0=OP.mult, op1=OP.mult,
                            accum_out=scores[:, jj : jj + 1],
                        )
                        nc.scalar.copy(t16[:, f, :], t[:, f, :])
                    bigs.append(t16)

                # --- softmax over [128 partitions x 32 cols]
                m_part = small_pool.tile([P, 1], f32, tag="m_part")
                nc.vector.reduce_max(m_part[:], scores[:], axis=AX.X)
                mT = psS.tile([1, P], f32, tag="psS")
                nc.tensor.transpose(mT[:], m_part[:], identity[:])
                negm = small_pool.tile([1, 1], f32, tag="negm")
                nc.vector.reduce_max(negm[:], mT[:], axis=AX.X, negate=True)
                negm_ps = psS.tile([P, 1], f32, tag="psS")
                nc.tensor.matmul(
                    negm_ps[:], ones_row[:], negm[:], start=True, stop=True
                )
                negm_b = small_pool.tile([P, 1], f32, tag="negm_b")
                nc.scalar.copy(negm_b[:], negm_ps[:])

                e = small_pool.tile([P, NT], f32, tag="e")
                z_part = small_pool.tile([P, 1], f32, tag="z_part")
                nc.scalar.activation(
                    e[:], scores[:], AF.Exp,
                    bias=negm_b[:], scale=1.0, accum_out=z_part[:],
                )
                z_ps = psS.tile([1, 1], f32, tag="psS")
                nc.tensor.matmul(z_ps[:], ones_col[:], z_part[:], start=True, stop=True)
                rz = small_pool.tile([1, 1], f32, tag="rz")
                nc.vector.reciprocal(rz[:], z_ps[:])
                rz_ps = psS.tile([P, 1], f32, tag="psS")
                nc.tensor.matmul(rz_ps[:], ones_row[:], rz[:], start=True, stop=True)
                rz_b = small_pool.tile([P, 1], f32, tag="rz_b")
                nc.scalar.copy(rz_b[:], rz_ps[:])

                # --- attn output, transposed to [32, 128] for contiguous rows
                eT = psT.tile([NT, P], f32, tag="eT")
                nc.tensor.transpose(eT[:], e[:], identity[:])
                attnT = small_pool.tile([NT, P], f32, tag="attnT")
                nc.scalar.mul(attnT[:], eT[:], rz_b[0:NT, :])
                nc.sync.dma_start(
                    out=attn_out[b].rearrange("(j p) -> j p", p=P), in_=attnT[:]
                )

                # --- context: accumulate e-weighted rows over all s-subtiles
                e16 = small_pool.tile([P, NT], bf16, tag="e16")
                nc.vector.tensor_copy(e16[:], e[:])
                ctx_ps = psC.tile([1, H], f32, tag="ctx_ps")
                for jb in range(NBIG):
                    for f in range(F):
                        jj = jb * F + f
                        st = jj == 0
                        sp = jj == NT - 1
                        for k in range(NH):
                            c = slice(k * 512, (k + 1) * 512)
                            nc.tensor.matmul(
                                ctx_ps[:, c],
                                e16[:, jj : jj + 1],
                                bigs[jb][:, f, c],
                                start=st, stop=sp,
                            )
                ctx_sb = small_pool.tile([1, H], f32, tag="ctx_sb")
                for k in range(NH):
                    c = slice(k * 512, (k + 1) * 512)
                    nc.scalar.mul(ctx_sb[:, c], ctx_ps[:, c], rz[:])
                nc.sync.dma_start(out=ctx_out[b : b + 1, :], in_=ctx_sb[:])

    if finalize:
        nc.finalize()
    return nc


def _get_program():
    if "nc" not in _CACHE:
        _CACHE["nc"] = build_program()
    return _CACHE["nc"]


def _install_ntff_hook_shim():
    """Provide antenv.axon_hooks if the image lacks it, so trace=True /
    BASS_TRACE=1 works (and doesn't crash run_bass_kernel_spmd)."""
    import sys
    import types

    try:
        import antenv.axon_hooks  # noqa: F401

        return
    except ImportError:
        pass
    hook = None
    try:
        from trn_agent_boot.trn_boot import _ntff_profile_via_ctypes

        hook = _ntff_profile_via_ctypes("/opt/axon/libaxon_pjrt.so")
    except Exception:
        pass
    mod = types.ModuleType("antenv.axon_hooks")
    state = {"hook": hook}
    mod.set_axon_ntff_profile_hook = lambda h: state.__setitem__("hook", h)
    mod.get_axon_ntff_profile_hook = lambda: state["hook"]
    sys.modules["antenv.axon_hooks"] = mod
    try:
        import antenv

        antenv.axon_hooks = mod
    except ImportError:
        pass


def kernel(decoder_state_t, encoder_outputs):
    _install_ntff_hook_shim()
    from concourse.bass_utils import run_bass_kernel_spmd

    nc = _get_program()
    dec = np.ascontiguousarray(np.asarray(decoder_state_t, dtype=np.float32))
    enc = np.ascontiguousarray(np.asarray(encoder_outputs, dtype=np.float32))
    assert dec.shape == (B, H) and enc.shape == (B, S, H)

    in_maps = []
    for c in range(NCORES):
        sl = slice(c * B_LOC, (c + 1) * B_LOC)
        in_maps.append({"dec": dec[sl], "enc": enc[sl]})

    res = run_bass_kernel_spmd(nc, in_maps, core_ids=list(range(NCORES)))
    _CACHE["last"] = res

    context = np.concatenate([r["context"] for r in res.results], axis=0)
    attn = np.concatenate([r["attn"] for r in res.results], axis=0)
    return (context, attn)


# revision 10
# speedup vs baseline: 1.2700x; 1.0145x over previous
"""Attention-pooling kernel for TRN2 (8 NeuronCores, SPMD batch-parallel).

Problem: b=32, s=4096, h=1024 fp32
    scores[b,s] = sum_h dec[b,h] * enc[b,s,h]
    attn        = softmax(scores, axis=-1)
    context[b,h]= sum_s attn[b,s] * enc[b,s,h]

Strategy: shard batch over 8 cores (4 batches/core). Per batch, the 16 MiB
encoder block is streamed into SBUF once and both contractions read it from
SBUF, so HBM traffic is the 64 MiB/core floor (~186 us at ~358 GB/s).
  - scores: DVE tensor_tensor_reduce (enc_tile * dec_bcast, accum over h)
  - softmax: free-dim reduces + small PE transposes / ones-matmuls for the
    cross-partition max/sum/broadcast steps
  - context: PE matmuls lhsT=e[:,j] (K=128 s-rows, M=1), rhs=enc tile
    (N=512), accumulated in PSUM over the 32 s-subtiles
"""

import numpy as np

B, S, H = 32, 4096, 1024
NCORES = 8
B_LOC = B // NCORES  # 4 batches per core
P = 128              # partitions
F = 2                # s-subtiles per DMA (1 MiB loads)
NT = S // P          # 32 s-subtiles per batch
NBIG = NT // F       # 16 DMA'd tiles per batch
NH = H // 512        # moving-dim chunks per matmul row

_CACHE = {}


def build_program(finalize=True):
    import concourse.mybir as mybir
    import concourse.tile as tile
    from concourse import bacc
    from concourse.masks import make_identity

    f32 = mybir.dt.float32
    AF = mybir.ActivationFunctionType
    AX = mybir.AxisListType
    OP = mybir.AluOpType

    nc = bacc.Bacc("TRN2", target_bir_lowering=False, debug=False)
    dec = nc.dram_tensor("dec", [B_LOC, H], f32, kind="ExternalInput")
    enc = nc.dram_tensor("enc", [B_LOC, S, H], f32, kind="ExternalInput")
    ctx_out = nc.dram_tensor("context", [B_LOC, H], f32, kind="ExternalOutput")
    attn_out = nc.dram_tensor("attn", [B_LOC, S], f32, kind="ExternalOutput")

    bf16 = mybir.dt.bfloat16

    with tile.TileContext(nc) as tc:
        with (
            tc.tile_pool(name="const", bufs=1) as const_pool,
            tc.tile_pool(name="encp", bufs=5) as enc_pool,
            tc.tile_pool(name="enc16p", bufs=NBIG + 10) as enc16_pool,
            tc.tile_pool(name="decp", bufs=2) as dec_pool,
            tc.tile_pool(name="scr", bufs=3) as scr_pool,
            tc.tile_pool(name="small", bufs=2) as small_pool,
            tc.tile_pool(name="psA", bufs=1, space="PSUM") as psA,
            tc.tile_pool(name="psT", bufs=1, space="PSUM") as psT,
            tc.tile_pool(name="psS", bufs=2, space="PSUM") as psS,
            tc.tile_pool(name="psC", bufs=1, space="PSUM") as psC,
        ):
            identity = const_pool.tile([P, P], f32)
            make_identity(nc, identity[:])
            ones_col = const_pool.tile([P, 1], f32)  # lhsT for partition-sum
            nc.vector.memset(ones_col[:], 1.0)
            ones_row = const_pool.tile([1, P], f32)  # lhsT for partition-bcast
            nc.vector.memset(ones_row[:], 1.0)

            for b in range(B_LOC):
                # --- broadcast dec[b] to all 128 partitions (PE ones-matmul)
                dec_row = dec_pool.tile([1, H], f32, tag="dec_row")
                nc.sync.dma_start(out=dec_row[:], in_=dec[b : b + 1, :])
                dec_ps = psA.tile([P, H], f32, tag="dec_ps")
                dec_b = dec_pool.tile([P, H], f32, tag="dec_b")
                for k in range(NH):
                    c = slice(k * 512, (k + 1) * 512)
                    nc.tensor.matmul(
                        dec_ps[:, c], ones_row[:], dec_row[:, c],
                        start=True, stop=True,
                    )
                    nc.scalar.copy(dec_b[:, c], dec_ps[:, c])

                # --- stream encoder tiles: fused dot-product on DVE for
                # scores (scalar_tensor_tensor with accum_out), bf16 copy on
                # ACT for the context matmuls (4x cheaper on PE than fp32's
                # LOW_HIGH two-pass mode; context error ~2e-3 scale-relative)
                scores = small_pool.tile([P, NT], f32, tag="scores")
                bigs = []
                for jb in range(NBIG):
                    t = enc_pool.tile([P, F, H], f32, tag="enc")
                    nc.sync.dma_start(
                        out=t[:],
                        in_=enc[b, jb * (F * P) : (jb + 1) * (F * P), :].rearrange(
                            "(f p) h -> p f h", p=P
                        ),
                    )
                    t16 = enc16_pool.tile([P, F, H], bf16, tag="enc16")
                    for f in range(F):
                        jj = jb * F + f
                        scr = scr_pool.tile([P, H], f32, tag="scr")
                        nc.vector.scalar_tensor_tensor(
                            out=scr[:], in0=t[:, f, :], scalar=1.0, in1=dec_b[:],
                            op0=OP.mult, op1=OP.mult,
                            accum_out=scores[:, jj : jj + 1],
                        )
                        nc.scalar.copy(t16[:, f, :], t[:, f, :])
                    bigs.append(t16)

                # --- softmax over [128 partitions x 32 cols]
                m_part = small_pool.tile([P, 1], f32, tag="m_part")
                nc.vector.reduce_max(m_part[:], scores[:], axis=AX.X)
                mT = psS.tile([1, P], f32, tag="psS")
                nc.tensor.transpose(mT[:], m_part[:], identity[:])
                negm = small_pool.tile([1, 1], f32, tag="negm")
                nc.vector.reduce_max(negm[:], mT[:], axis=AX.X, negate=True)
                negm_ps = psS.tile([P, 1], f32, tag="psS")
                nc.tensor.matmul(
                    negm_ps[:], ones_row[:], negm[:], start=True, stop=True
                )
                negm_b = small_pool.tile([P, 1], f32, tag="negm_b")
                nc.scalar.copy(negm_b[:], negm_ps[:])

                e = small_pool.tile([P, NT], f32, tag="e")
                z_part = small_pool.tile([P, 1], f32, tag="z_part")
                nc.scalar.activation(
                    e[:], scores[:], AF.Exp,
                    bias=negm_b[:], scale=1.0, accum_out=z_part[:],
                )
                z_ps = psS.tile([1, 1], f32, tag="psS")
                nc.tensor.matmul(z_ps[:], ones_col[:], z_part[:], start=True, stop=True)
                rz = small_pool.tile([1, 1], f32, tag="rz")
                nc.vector.reciprocal(rz[:], z_ps[:])
                rz_ps = psS.tile([P, 1], f32, tag="psS")
                nc.tensor.matmul(rz_ps[:], ones_row[:], rz[:], start=True, stop=True)
                rz_b = small_pool.tile([P, 1], f32, tag="rz_b")
                nc.scalar.copy(rz_b[:], rz_ps[:])

                # --- attn output, transposed to [32, 128] for contiguous rows
                eT = psT.tile([NT, P], f32, tag="eT")
                nc.tensor.transpose(eT[:], e[:], identity[:])
                attnT = small_pool.tile([NT, P], f32, tag="attnT")
                nc.scalar.mul(attnT[:], eT[:], rz_b[0:NT, :])
                nc.sync.dma_start(
                    out=attn_out[b].rearrange("(j p) -> j p", p=P), in_=attnT[:]
                )

                # --- context: accumulate e-weighted rows over all s-subtiles
                e16 = small_pool.tile([P, NT], bf16, tag="e16")
                nc.vector.tensor_copy(e16[:], e[:])
                ctx_ps = psC.tile([1, H], f32, tag="ctx_ps")
                for jb in range(NBIG):
                    for f in range(F):
                        jj = jb * F + f
                        st = jj == 0
                        sp = jj == NT - 1
                        for k in range(NH):
                            c = slice(k * 512, (k + 1) * 512)
                            nc.tensor.matmul(
                                ctx_ps[:, c],
                                e16[:, jj : jj + 1],
                                bigs[jb][:, f, c],
                                start=st, stop=sp,
                            )
                ctx_sb = small_pool.tile([1, H], f32, tag="ctx_sb")
                for k in range(NH):
                    c = slice(k * 512, (k + 1) * 512)
                    nc.scalar.mul(ctx_sb[:, c], ctx_ps[:, c], rz[:])
                nc.sync.dma_start(out=ctx_out[b : b + 1, :], in_=ctx_sb[:])

    if finalize:
        nc.finalize()
    return nc


def _get_program():
    if "nc" not in _CACHE:
        _CACHE["nc"] = build_program()
    return _CACHE["nc"]


def _install_ntff_hook_shim():
    """Provide antenv.axon_hooks if the image lacks it, so trace=True /
    BASS_TRACE=1 works (and doesn't crash run_bass_kernel_spmd)."""
    import sys
    import types

    try:
        import antenv.axon_hooks  # noqa: F401

        return
    except ImportError:
        pass
    hook = None
    try:
        from trn_agent_boot.trn_boot import _ntff_profile_via_ctypes

        hook = _ntff_profile_via_ctypes("/opt/axon/libaxon_pjrt.so")
    except Exception:
        pass
    mod = types.ModuleType("antenv.axon_hooks")
    state = {"hook": hook}
    mod.set_axon_ntff_profile_hook = lambda h: state.__setitem__("hook", h)
    mod.get_axon_ntff_profile_hook = lambda: state["hook"]
    sys.modules["antenv.axon_hooks"] = mod
    try:
        import antenv

        antenv.axon_hooks = mod
    except ImportError:
        pass


def kernel(decoder_state_t, encoder_outputs):
    _install_ntff_hook_shim()
    from concourse.bass_utils import run_bass_kernel_spmd

    nc = _get_program()
    dec = np.ascontiguousarray(np.asarray(decoder_state_t, dtype=np.float32))
    enc = np.ascontiguousarray(np.asarray(encoder_outputs, dtype=np.float32))
    assert dec.shape == (B, H) and enc.shape == (B, S, H)

    in_maps = []
    for c in range(NCORES):
        sl = slice(c * B_LOC, (c + 1) * B_LOC)
        in_maps.append({"dec": dec[sl], "enc": enc[sl]})

    res = run_bass_kernel_spmd(nc, in_maps, core_ids=list(range(NCORES)))
    _CACHE["last"] = res

    context = np.concatenate([r["context"] for r in res.results], axis=0)
    attn = np.concatenate([r["attn"] for r in res.results], axis=0)
    return (context, attn)


# revision 12
# speedup vs baseline: 1.4250x; 1.1220x over previous
"""Attention-pooling kernel for TRN2 (8 NeuronCores, SPMD batch-parallel).

Problem: b=32, s=4096, h=1024 fp32
    scores[b,s] = sum_h dec[b,h] * enc[b,s,h]
    attn        = softmax(scores, axis=-1)
    context[b,h]= sum_s attn[b,s] * enc[b,s,h]

Strategy: shard batch over 8 cores (4 batches/core). Per batch, the 16 MiB
encoder block is streamed into SBUF once and both contractions read it from
SBUF, so HBM traffic is the 64 MiB/core floor (~186 us at ~358 GB/s).
  - scores: DVE tensor_tensor_reduce (enc_tile * dec_bcast, accum over h)
  - softmax: free-dim reduces + small PE transposes / ones-matmuls for the
    cross-partition max/sum/broadcast steps
  - context: PE matmuls lhsT=e[:,j] (K=128 s-rows, M=1), rhs=enc tile
    (N=512), accumulated in PSUM over the 32 s-subtiles
"""

import numpy as np

B, S, H = 32, 4096, 1024
NCORES = 8
B_LOC = B // NCORES  # 4 batches per core
P = 128              # partitions
F = 2                # s-subtiles per DMA (1 MiB loads)
NT = S // P          # 32 s-subtiles per batch
NBIG = NT // F       # 16 DMA'd tiles per batch
NH = H // 512        # moving-dim chunks per matmul row

_CACHE = {}


def build_program(finalize=True):
    import concourse.mybir as mybir
    import concourse.tile as tile
    from concourse import bacc
    from concourse.masks import make_identity

    f32 = mybir.dt.float32
    AF = mybir.ActivationFunctionType
    AX = mybir.AxisListType
    OP = mybir.AluOpType

    nc = bacc.Bacc("TRN2", target_bir_lowering=False, debug=False)
    dec = nc.dram_tensor("dec", [B_LOC, H], f32, kind="ExternalInput")
    enc = nc.dram_tensor("enc", [B_LOC, S, H], f32, kind="ExternalInput")
    ctx_out = nc.dram_tensor("context", [B_LOC, H], f32, kind="ExternalOutput")
    attn_out = nc.dram_tensor("attn", [B_LOC, S], f32, kind="ExternalOutput")

    bf16 = mybir.dt.bfloat16

    with tile.TileContext(nc) as tc:
        with (
            tc.tile_pool(name="const", bufs=1) as const_pool,
            tc.tile_pool(name="encp", bufs=5) as enc_pool,
            tc.tile_pool(name="enc16p", bufs=NBIG + 10) as enc16_pool,
            tc.tile_pool(name="decp", bufs=2) as dec_pool,
            tc.tile_pool(name="scr", bufs=3) as scr_pool,
            tc.tile_pool(name="small", bufs=2) as small_pool,
            tc.tile_pool(name="psA", bufs=1, space="PSUM") as psA,
            tc.tile_pool(name="psT", bufs=1, space="PSUM") as psT,
            tc.tile_pool(name="psS", bufs=2, space="PSUM") as psS,
            tc.tile_pool(name="psC", bufs=1, space="PSUM") as psC,
        ):
            identity = const_pool.tile([P, P], f32)
            make_identity(nc, identity[:])
            ones_col = const_pool.tile([P, 1], f32)  # lhsT for partition-sum
            nc.vector.memset(ones_col[:], 1.0)
            ones_row = const_pool.tile([1, P], f32)  # lhsT for partition-bcast
            nc.vector.memset(ones_row[:], 1.0)

            for b in range(B_LOC):
                # --- broadcast dec[b] to all 128 partitions (PE ones-matmul)
                dec_row = dec_pool.tile([1, H], f32, tag="dec_row")
                nc.sync.dma_start(out=dec_row[:], in_=dec[b : b + 1, :])
                dec_ps = psA.tile([P, H], f32, tag="dec_ps")
                dec_b = dec_pool.tile([P, H], f32, tag="dec_b")
                for k in range(NH):
                    c = slice(k * 512, (k + 1) * 512)
                    nc.tensor.matmul(
                        dec_ps[:, c], ones_row[:], dec_row[:, c],
                        start=True, stop=True,
                    )
                    nc.scalar.copy(dec_b[:, c], dec_ps[:, c])

                # --- stream encoder tiles: fused dot-product on DVE for
                # scores (scalar_tensor_tensor with accum_out), bf16 copy on
                # ACT for the context matmuls (4x cheaper on PE than fp32's
                # LOW_HIGH two-pass mode; context error ~2e-3 scale-relative)
                scores = small_pool.tile([P, NT], f32, tag="scores")
                bigs = []
                for jb in range(NBIG):
                    t = enc_pool.tile([P, F, H], f32, tag="enc")
                    nc.sync.dma_start(
                        out=t[:],
                        in_=enc[b, jb * (F * P) : (jb + 1) * (F * P), :].rearrange(
                            "(f p) h -> p f h", p=P
                        ),
                    )
                    t16 = enc16_pool.tile([P, F, H], bf16, tag="enc16")
                    for f in range(F):
                        jj = jb * F + f
                        scr = scr_pool.tile([P, H], f32, tag="scr")
                        nc.vector.scalar_tensor_tensor(
                            out=scr[:], in0=t[:, f, :], scalar=1.0, in1=dec_b[:],
                            op0=OP.mult, op1=OP.mult,
                            accum_out=scores[:, jj : jj + 1],
                        )
                        nc.scalar.copy(t16[:, f, :], t[:, f, :])
                    bigs.append(t16)

                # --- softmax over [128 partitions x 32 cols]
                m_part = small_pool.tile([P, 1], f32, tag="m_part")
                nc.vector.reduce_max(m_part[:], scores[:], axis=AX.X)
                mT = psS.tile([1, P], f32, tag="psS")
                nc.tensor.transpose(mT[:], m_part[:], identity[:])
                negm = small_pool.tile([1, 1], f32, tag="negm")
                nc.vector.reduce_max(negm[:], mT[:], axis=AX.X, negate=True)
                negm_ps = psS.tile([P, 1], f32, tag="psS")
                nc.tensor.matmul(
                    negm_ps[:], ones_row[:], negm[:], start=True, stop=True
                )
                negm_b = small_pool.tile([P, 1], f32, tag="negm_b")
                nc.scalar.copy(negm_b[:], negm_ps[:])

                e = small_pool.tile([P, NT], f32, tag="e")
                z_part = small_pool.tile([P, 1], f32, tag="z_part")
                nc.scalar.activation(
                    e[:], scores[:], AF.Exp,
                    bias=negm_b[:], scale=1.0, accum_out=z_part[:],
                )
                z_ps = psS.tile([1, 1], f32, tag="psS")
                nc.tensor.matmul(z_ps[:], ones_col[:], z_part[:], start=True, stop=True)
                rz = small_pool.tile([1, 1], f32, tag="rz")
                nc.vector.reciprocal(rz[:], z_ps[:])
                rz_ps = psS.tile([P, 1], f32, tag="psS")
                nc.tensor.matmul(rz_ps[:], ones_row[:], rz[:], start=True, stop=True)
                rz_b = small_pool.tile([P, 1], f32, tag="rz_b")
                nc.scalar.copy(rz_b[:], rz_ps[:])

                # --- attn output, transposed to [32, 128] for contiguous rows
                eT = psT.tile([NT, P], f32, tag="eT")
                nc.tensor.transpose(eT[:], e[:], identity[:])
                attnT = small_pool.tile([NT, P], f32, tag="attnT")
                nc.scalar.mul(attnT[:], eT[:], rz_b[0:NT, :])
                # output DMAs go on ACT's HWDGE ring: on the SP ring they'd
                # head-of-line-block the next batch's input streaming
                nc.scalar.dma_start(
                    out=attn_out[b].rearrange("(j p) -> j p", p=P), in_=attnT[:]
                )

                # --- context: accumulate e-weighted rows over all s-subtiles
                e16 = small_pool.tile([P, NT], bf16, tag="e16")
                nc.vector.tensor_copy(e16[:], e[:])
                ctx_ps = psC.tile([1, H], f32, tag="ctx_ps")
                for jb in range(NBIG):
                    for f in range(F):
                        jj = jb * F + f
                        st = jj == 0
                        sp = jj == NT - 1
                        for k in range(NH):
                            c = slice(k * 512, (k + 1) * 512)
                            nc.tensor.matmul(
                                ctx_ps[:, c],
                                e16[:, jj : jj + 1],
                                bigs[jb][:, f, c],
                                start=st, stop=sp,
                            )
                ctx_sb = small_pool.tile([1, H], f32, tag="ctx_sb")
                for k in range(NH):
                    c = slice(k * 512, (k + 1) * 512)
                    nc.scalar.mul(ctx_sb[:, c], ctx_ps[:, c], rz[:])
                nc.scalar.dma_start(out=ctx_out[b : b + 1, :], in_=ctx_sb[:])

    if finalize:
        nc.finalize()
    return nc


def _get_program():
    if "nc" not in _CACHE:
        _CACHE["nc"] = build_program()
    return _CACHE["nc"]


def _install_ntff_hook_shim():
    """Provide antenv.axon_hooks if the image lacks it, so trace=True /
    BASS_TRACE=1 works (and doesn't crash run_bass_kernel_spmd)."""
    import sys
    import types

    try:
        import antenv.axon_hooks  # noqa: F401

        return
    except ImportError:
        pass
    hook = None
    try:
        from trn_agent_boot.trn_boot import _ntff_profile_via_ctypes

        hook = _ntff_profile_via_ctypes("/opt/axon/libaxon_pjrt.so")
    except Exception:
        pass
    mod = types.ModuleType("antenv.axon_hooks")
    state = {"hook": hook}
    mod.set_axon_ntff_profile_hook = lambda h: state.__setitem__("hook", h)
    mod.get_axon_ntff_profile_hook = lambda: state["hook"]
    sys.modules["antenv.axon_hooks"] = mod
    try:
        import antenv

        antenv.axon_hooks = mod
    except ImportError:
        pass


def kernel(decoder_state_t, encoder_outputs):
    _install_ntff_hook_shim()
    from concourse.bass_utils import run_bass_kernel_spmd

    nc = _get_program()
    dec = np.ascontiguousarray(np.asarray(decoder_state_t, dtype=np.float32))
    enc = np.ascontiguousarray(np.asarray(encoder_outputs, dtype=np.float32))
    assert dec.shape == (B, H) and enc.shape == (B, S, H)

    in_maps = []
    for c in range(NCORES):
        sl = slice(c * B_LOC, (c + 1) * B_LOC)
        in_maps.append({"dec": dec[sl], "enc": enc[sl]})

    res = run_bass_kernel_spmd(nc, in_maps, core_ids=list(range(NCORES)))
    _CACHE["last"] = res

    context = np.concatenate([r["context"] for r in res.results], axis=0)
    attn = np.concatenate([r["attn"] for r in res.results], axis=0)
    return (context, attn)
